# revision 25
# baseline (speedup 1.0000x reference)
"""GIN-style GNN message passing on 8 TRN2 NeuronCores.

Pipeline (per core, nodes sharded by graph id so pooling is local):
  phase 1: edge aggregation  agg[dst] += x[src]
      - edges bucketed by (dst node-tile t, src block b) on host, padded to
        groups of 128; src rows gathered from HBM via gpsimd dma_gather
        (bf16, 256B rows); segment-sum via one-hot matmul into PSUM,
        accumulated into a feature-major aggT SBUF tile.
  phase 2: h = relu(relu((x+agg) @ w1 + b1) @ w2 + b2), pooled per graph
      via one-hot matmul, then the small MLP head + log_softmax.

The bass program is identical across the 8 cores (SPMD); all data-dependent
structure (bucket sizes) is made uniform by padding to the max over cores.
"""
import numpy as np
import ml_dtypes

import concourse.bacc as bacc
import concourse.tile as tile
from concourse import mybir
from concourse.bass_utils import run_bass_kernel_spmd
from concourse.library_config import mlp as mlp_lib

P = 128
F = 128
HID = 128
NCLS = 10
NCORES = 8
CALL_G = 8  # groups per dma_gather call; 1024 descs = HW ring cap (hard)
RING_BYTES = 16384  # dynamic_dma_scratch_size (runtime ignores larger)
GBUF_BUFS = 8

FP32 = mybir.dt.float32
BF16 = mybir.dt.bfloat16
I16 = mybir.dt.int16


def build_program(NT, G_tb, Np, blocks, GPG, rep=1):
    """Build the SPMD bass program.

    NT: node tiles per core; G_tb: [NT, NBLK] groups per bucket; Np: NT*P;
    blocks: src block sizes (each <= 32767 rows); GPG: graphs per core.
    """
    NBLK = len(blocks)
    bstart = [0]
    for bs in blocks:
        bstart.append(bstart[-1] + bs)
    TOT_G = int(G_tb.sum())
    nc = bacc.Bacc("TRN2", target_bir_lowering=False, debug=False,
                   num_swdge_queues=4, dynamic_dma_scratch_size=RING_BYTES)

    xb_t = nc.declare_dram_parameter("xb", [bstart[-1], F], BF16, isOutput=False)
    idx_t = nc.declare_dram_parameter("idx", [P, TOT_G * 8], I16, isOutput=False)
    dst_t = nc.declare_dram_parameter("dstc", [P, TOT_G], BF16, isOutput=False)
    xt_t = nc.declare_dram_parameter("xt", [P, Np], FP32, isOutput=False)
    bc_t = nc.declare_dram_parameter("bc", [P, NT], FP32, isOutput=False)
    w1_t = nc.declare_dram_parameter("w1", [F, HID], FP32, isOutput=False)
    b1_t = nc.declare_dram_parameter("b1", [HID, 1], FP32, isOutput=False)
    w2_t = nc.declare_dram_parameter("w2", [HID, HID], FP32, isOutput=False)
    b2_t = nc.declare_dram_parameter("b2", [HID, 1], FP32, isOutput=False)
    l1w_t = nc.declare_dram_parameter("l1w", [HID, HID], FP32, isOutput=False)
    l1b_t = nc.declare_dram_parameter("l1b", [HID, 1], FP32, isOutput=False)
    l2w_t = nc.declare_dram_parameter("l2w", [HID, NCLS], FP32, isOutput=False)
    l2b_t = nc.declare_dram_parameter("l2b", [GPG, NCLS], FP32, isOutput=False)
    out_t = nc.declare_dram_parameter("out", [GPG, NCLS], FP32, isOutput=True)

    import ml_dtypes as _mld
    iota_c = nc.inline_tensor(
        np.tile(np.arange(P, dtype=_mld.bfloat16), (P, CALL_G)), name="iota128")
    iotag_c = nc.inline_tensor(
        np.tile(np.arange(GPG, dtype=np.float32), (P, 4)), name="iotag")
    ident_c = nc.inline_tensor(np.eye(P, dtype=np.float32), name="ident")

    with tile.TileContext(nc) as tc:
        nc.gpsimd.load_library(mlp_lib)
        with tc.tile_pool(name="const", bufs=1) as cpool, \
             tc.tile_pool(name="agg", bufs=NT) as apool, \
             tc.tile_pool(name="gbuf", bufs=GBUF_BUFS) as gpool, \
             tc.tile_pool(name="ibuf", bufs=2) as ipool, \
             tc.tile_pool(name="oh", bufs=4) as ohpool, \
             tc.tile_pool(name="p2s", bufs=3) as spool, \
             tc.tile_pool(name="psum1", bufs=3, space="PSUM") as p1, \
             tc.tile_pool(name="psum2", bufs=2, space="PSUM") as p2, \
             tc.tile_pool(name="psumg", bufs=1, space="PSUM") as pg:

            iota_sb = cpool.tile([P, CALL_G * P], BF16)
            nc.sync.dma_start(out=iota_sb[:], in_=iota_c[:])
            iotag_sb = cpool.tile([P, 4 * GPG], FP32)
            nc.sync.dma_start(out=iotag_sb[:], in_=iotag_c[:])
            ident_sb = cpool.tile([P, P], FP32)
            nc.sync.dma_start(out=ident_sb[:], in_=ident_c[:])
            dstc_sb = cpool.tile([P, TOT_G], BF16)
            nc.sync.dma_start(out=dstc_sb[:], in_=dst_t[:])
            bc_sb = cpool.tile([P, NT], FP32)
            nc.sync.dma_start(out=bc_sb[:], in_=bc_t[:])
            w1_sb = cpool.tile([F, HID], FP32)
            nc.sync.dma_start(out=w1_sb[:], in_=w1_t[:])
            b1_sb = cpool.tile([HID, 1], FP32)
            nc.sync.dma_start(out=b1_sb[:], in_=b1_t[:])
            w2_sb = cpool.tile([HID, HID], FP32)
            nc.sync.dma_start(out=w2_sb[:], in_=w2_t[:])
            b2_sb = cpool.tile([HID, 1], FP32)
            nc.sync.dma_start(out=b2_sb[:], in_=b2_t[:])
            l1w_sb = cpool.tile([HID, HID], FP32)
            nc.sync.dma_start(out=l1w_sb[:], in_=l1w_t[:])
            l1b_sb = cpool.tile([HID, 1], FP32)
            nc.sync.dma_start(out=l1b_sb[:], in_=l1b_t[:])
            l2w_sb = cpool.tile([HID, NCLS], FP32)
            nc.sync.dma_start(out=l2w_sb[:], in_=l2w_t[:])
            l2b_sb = cpool.tile([GPG, NCLS], FP32)
            nc.sync.dma_start(out=l2b_sb[:], in_=l2b_t[:])

            # repeated body (rep>1 used only for benchmarking)
            for _rep in range(rep):
                aggts = []
                for _t in range(NT):
                    agg_tile = apool.tile([P, P], FP32, tag="aggt")
                    aggts.append(agg_tile)

                # ---- phase 1 + interleaved phase 2 ----
                gacc = pg.tile([GPG, HID], FP32)
                CH = 4  # phase-2 tiles per chunk; rhs width CH*P = 512

                def emit_phase2(c):
                    """MLP + pooling for node tiles [c*CH, min(NT,(c+1)*CH))."""
                    t0 = c * CH
                    w = min(CH, NT - t0) * P
                    xt_sb = spool.tile([P, CH * P], FP32, tag="xt")
                    nc.sync.dma_start(out=xt_sb[:, :w],
                                      in_=xt_t[:, t0 * P:t0 * P + w])
                    hin = spool.tile([P, CH * P], FP32, tag="hin")
                    for i in range(w // P):
                        nc.vector.tensor_add(
                            out=hin[:, i * P:(i + 1) * P],
                            in0=xt_sb[:, i * P:(i + 1) * P],
                            in1=aggts[t0 + i][:])
                    ps1 = p2.tile([P, CH * P], FP32, tag="wide")
                    nc.tensor.matmul(out=ps1[:, :w], lhsT=w1_sb[:],
                                     rhs=hin[:, :w], start=True, stop=True)
                    h1 = spool.tile([P, CH * P], FP32, tag="h1")
                    nc.scalar.activation(out=h1[:, :w], in_=ps1[:, :w],
                                         func=mybir.ActivationFunctionType.Relu,
                                         bias=b1_sb[:, 0:1])
                    ps2 = p2.tile([P, CH * P], FP32, tag="wide")
                    nc.tensor.matmul(out=ps2[:, :w], lhsT=w2_sb[:],
                                     rhs=h1[:, :w], start=True, stop=True)
                    h2 = spool.tile([P, CH * P], FP32, tag="h2")
                    nc.scalar.activation(out=h2[:, :w], in_=ps2[:, :w],
                                         func=mybir.ActivationFunctionType.Relu,
                                         bias=b2_sb[:, 0:1])
                    nch = w // P
                    ohg = ohpool.tile([P, 4, GPG], FP32, tag="ohg")
                    nc.vector.tensor_tensor(
                        out=ohg[:, :nch, :], in0=iotag_sb[:, :nch * GPG],
                        in1=bc_sb[:, t0:t0 + nch].unsqueeze(2)
                            .broadcast_to([P, nch, GPG]),
                        op=mybir.AluOpType.is_equal)
                    for i in range(nch):
                        t = t0 + i
                        ps3 = p2.tile([P, P], FP32, tag="ps")
                        nc.tensor.transpose(out=ps3[:],
                                            in_=h2[:, i * P:(i + 1) * P],
                                            identity=ident_sb[:])
                        h2t = spool.tile([P, P], FP32, tag="h2t")
                        nc.vector.tensor_copy(out=h2t[:], in_=ps3[:])
                        nc.tensor.matmul(out=gacc[:], lhsT=ohg[:, i, :],
                                         rhs=h2t[:],
                                         start=(t == 0), stop=(t == NT - 1))

                max_gb_pass = max(int(G_tb[:, b].sum()) for b in range(NBLK))
                ci = 0  # global call counter (queue = ci % 4)
                goff = 0  # global group index (b-major order, matches host layout)
                p2_next = 0  # next phase-2 chunk to emit (during last pass)
                for b in range(NBLK):
                    last = b == NBLK - 1
                    seq = []  # (t, j, glast) for each group of this pass
                    for t in range(NT):
                        gb = int(G_tb[t, b])
                        for j in range(gb):
                            seq.append((t, j, gb))
                    nb = len(seq)
                    idx_sb = ipool.tile([P, max_gb_pass * 8], I16, tag="idx")
                    nc.sync.dma_start(
                        out=idx_sb[:, :nb * 8],
                        in_=idx_t[:, goff * 8:(goff + nb) * 8])
                    # chunk into gather calls
                    for c0 in range(0, nb, CALL_G):
                        ng = min(CALL_G, nb - c0)
                        q = ci % 4
                        ci += 1
                        gbuf = gpool.tile([P, CALL_G, F], BF16, tag="gbuf")
                        nc.gpsimd.dma_gather(
                            out_ap=gbuf[:, :ng, :],
                            in_ap=xb_t[bstart[b]:bstart[b + 1], :],
                            idxs_ap=idx_sb[:, c0 * 8:(c0 + ng) * 8],
                            num_idxs=ng * P,
                            num_idxs_reg=ng * P,
                            elem_size=F,
                            queue_num=q,
                        )
                        seq_c = seq[c0:c0 + ng]
                        oh = ohpool.tile([P, CALL_G, P], BF16, tag="oh")
                        g0 = goff + c0
                        nc.vector.tensor_tensor(
                            out=oh[:, :ng, :], in0=iota_sb[:, :ng * P],
                            in1=dstc_sb[:, g0:g0 + ng].unsqueeze(2)
                                .broadcast_to([P, ng, P]),
                            op=mybir.AluOpType.is_equal)
                        for s, (t, j, gb) in enumerate(seq_c):
                            if j == 0:
                                ps = p1.tile([P, P], FP32, tag="p1")
                                cur_ps = ps
                            else:
                                ps = cur_ps
                            nc.tensor.matmul(
                                out=ps[:], lhsT=gbuf[:, s, :], rhs=oh[:, s, :],
                                start=(j == 0), stop=(j == gb - 1))
                            if j == gb - 1:
                                sl = aggts[t][:]
                                if b == 0:
                                    nc.scalar.copy(out=sl, in_=ps[:])
                                else:
                                    nc.vector.tensor_add(out=sl, in0=sl, in1=ps[:])
                        if last:
                            # tiles strictly before t_next are fully aggregated
                            t_next = seq[c0 + ng][0] if c0 + ng < nb else NT
                            while (p2_next + 1) * CH <= t_next:
                                emit_phase2(p2_next)
                                p2_next += 1
                    goff += nb
                while p2_next * CH < NT:
                    emit_phase2(p2_next)
                    p2_next += 1

                # ---- head MLP + log_softmax ----
                g_sb = spool.tile([GPG, HID], FP32, tag="g")
                nc.scalar.copy(out=g_sb[:], in_=gacc[:])
                psg = p2.tile([HID, GPG], FP32, tag="ps")
                nc.tensor.transpose(out=psg[:], in_=g_sb[:],
                                    identity=ident_sb[:GPG, :GPG])
                gt = spool.tile([HID, GPG], FP32, tag="gt")
                nc.vector.tensor_copy(out=gt[:], in_=psg[:])
                ps4 = p2.tile([HID, GPG], FP32, tag="ps")
                nc.tensor.matmul(out=ps4[:], lhsT=l1w_sb[:], rhs=gt[:],
                                 start=True, stop=True)
                g1 = spool.tile([HID, GPG], FP32, tag="g1")
                nc.scalar.activation(out=g1[:], in_=ps4[:],
                                     func=mybir.ActivationFunctionType.Relu,
                                     bias=l1b_sb[:, 0:1])
                ps5 = p2.tile([GPG, NCLS], FP32, tag="ps")
                nc.tensor.matmul(out=ps5[:], lhsT=g1[:], rhs=l2w_sb[:],
                                 start=True, stop=True)
                logits = spool.tile([GPG, NCLS], FP32, tag="lg")
                nc.vector.tensor_tensor(out=logits[:], in0=ps5[:], in1=l2b_sb[:],
                                        op=mybir.AluOpType.add)
                mx = spool.tile([GPG, 1], FP32, tag="mx")
                nc.vector.tensor_reduce(out=mx[:], in_=logits[:],
                                        axis=mybir.AxisListType.X,
                                        op=mybir.AluOpType.max)
                sh = spool.tile([GPG, NCLS], FP32, tag="sh")
                nc.vector.tensor_scalar(out=sh[:], in0=logits[:],
                                        scalar1=mx[:, 0:1], scalar2=None,
                                        op0=mybir.AluOpType.subtract)
                ex = spool.tile([GPG, NCLS], FP32, tag="ex")
                ssum = spool.tile([GPG, 1], FP32, tag="ssum")
                nc.scalar.activation(out=ex[:], in_=sh[:],
                                     func=mybir.ActivationFunctionType.Exp,
                                     accum_out=ssum[:])
                lse = spool.tile([GPG, 1], FP32, tag="lse")
                nc.scalar.activation(out=lse[:], in_=ssum[:],
                                     func=mybir.ActivationFunctionType.Ln)
                res = spool.tile([GPG, NCLS], FP32, tag="res")
                nc.vector.tensor_scalar(out=res[:], in0=sh[:],
                                        scalar1=lse[:, 0:1], scalar2=None,
                                        op0=mybir.AluOpType.subtract)
                nc.sync.dma_start(out=out_t[:], in_=res[:])

    nc.compile()
    return nc


def assign_graphs(batch, dst, n_graphs, gpg):
    """LPT bin-packing of graphs onto cores by edge count, node-capped.

    Returns core_graphs [NCORES, gpg] (graph ids per core, sorted) such
    that per-core edge totals are balanced and node counts fit min NT.
    """
    gnodes = np.bincount(batch, minlength=n_graphs)
    gedges = np.bincount(batch[dst], minlength=n_graphs)
    node_cap = int(np.ceil(gnodes.sum() / NCORES / P)) * P  # target NT
    order = np.argsort(-gedges, kind="stable")
    ce = np.zeros(NCORES, np.int64)
    cn = np.zeros(NCORES, np.int64)
    cg = np.zeros(NCORES, np.int64)
    out = [[] for _ in range(NCORES)]
    for g in order:
        # least-loaded (edges) core with node+graph capacity
        best, be = -1, None
        for c in range(NCORES):
            if cg[c] < gpg and cn[c] + gnodes[g] <= node_cap:
                if be is None or ce[c] < be:
                    best, be = c, ce[c]
        if best < 0:  # node cap infeasible; fall back to graph-count only
            best = min((c for c in range(NCORES) if cg[c] < gpg),
                       key=lambda c: ce[c])
        out[best].append(g)
        ce[best] += gedges[g]
        cn[best] += gnodes[g]
        cg[best] += 1
    return [np.sort(np.array(gs, np.int64)) for gs in out]


def prep_inputs(x, edge_index, batch, conv_w1, conv_b1, conv_w2, conv_b2,
                lin1_w, lin1_b, lin2_w, lin2_b, n_graphs, blk, nblk):
    """Host-side sharding: returns (in_maps, NT, G_tb, Np, GPG, core_graphs).

    blk: int (uniform block size, nblk blocks) or list of block sizes.
    """
    blocks = [blk] * nblk if isinstance(blk, int) else list(blk)
    nblk = len(blocks)
    bstart = np.concatenate([[0], np.cumsum(blocks)])
    n_nodes = x.shape[0]
    x = np.asarray(x, np.float32)
    batch = np.asarray(batch, np.int64)
    src = np.asarray(edge_index[0], np.int64)
    dst = np.asarray(edge_index[1], np.int64)
    gpg = n_graphs // NCORES

    gbounds = np.concatenate(
        [[0], np.cumsum(np.bincount(batch, minlength=n_graphs))])
    core_graphs = assign_graphs(batch, dst, n_graphs, gpg)

    # per-node (core, local position); graphs keep contiguous node runs
    node_core = np.empty(n_nodes, np.int32)
    node_local = np.empty(n_nodes, np.int64)
    core_nodes = []  # global node ids per core, in local order
    counts = np.zeros(NCORES, np.int64)
    for c in range(NCORES):
        ids = np.concatenate([np.arange(gbounds[g], gbounds[g + 1])
                              for g in core_graphs[c]])
        core_nodes.append(ids)
        node_core[ids] = c
        node_local[ids] = np.arange(len(ids))
        counts[c] = len(ids)
    NT = max(1, int(np.ceil(counts.max() / P)))
    Np = NT * P

    core = node_core[dst].astype(np.int64)
    nlocal = node_local[dst]
    tt = nlocal // P
    dl = nlocal % P
    bb = np.searchsorted(bstart, src, side="right") - 1
    sl = src - bstart[bb]

    key = (core * NT + tt) * nblk + bb
    cnt = np.bincount(key, minlength=NCORES * NT * nblk).reshape(NCORES, NT, nblk)
    G_tb = np.ceil(cnt.max(axis=0) / P).astype(np.int64)  # [NT, nblk]
    G_tb[:, 0] = np.maximum(G_tb[:, 0], 1)
    TOT_G = int(G_tb.sum())

    # padded slot layout, b-major then t-major (must match device emission)
    slot_sizes = (G_tb.T * P).reshape(-1)  # [nblk*NT], order (b, t)
    slot_off = np.concatenate([[0], np.cumsum(slot_sizes)])[:-1].reshape(nblk, NT)
    total_slots = TOT_G * P

    # bf16 x table, padded rows
    xpad = np.zeros((int(bstart[-1]), F), np.float32)
    xpad[:n_nodes] = x
    xb = xpad.astype(ml_dtypes.bfloat16)

    in_maps = []
    for c in range(NCORES):
        m = core == c
        sl_c, dl_c, tt_c, bb_c = sl[m], dl[m], tt[m], bb[m]
        order = np.lexsort((sl_c, tt_c, bb_c))
        sl_c, dl_c, tt_c, bb_c = (sl_c[order], dl_c[order], tt_c[order],
                                  bb_c[order])
        # rank within bucket (edges sorted by (b, t); buckets contiguous)
        bucket = bb_c * NT + tt_c
        changes = np.concatenate([[True], bucket[1:] != bucket[:-1]])
        idx_in_run = np.arange(len(bucket)) - \
            np.maximum.accumulate(np.where(changes, np.arange(len(bucket)), 0))
        pos = slot_off[bb_c, tt_c] + idx_in_run

        SL = np.zeros(total_slots, np.int16)
        DL = np.full(total_slots, 255.0, np.float32)
        SL[pos] = sl_c.astype(np.int16)
        DL[pos] = dl_c.astype(np.float32)

        idx_arr = np.tile(SL.reshape(-1, 16).T, (8, 1)).astype(np.int16)
        dst_arr = DL.reshape(TOT_G, P).T.astype(ml_dtypes.bfloat16)

        ids, cn = core_nodes[c], counts[c]
        xt = np.zeros((P, Np), np.float32)
        xt[:, :cn] = x[ids].T
        g2l = np.full(n_graphs, 255, np.int64)
        g2l[core_graphs[c]] = np.arange(gpg)
        bc = np.full(Np, 255.0, np.float32)
        bc[:cn] = g2l[batch[ids]].astype(np.float32)
        bc = bc.reshape(NT, P).T.copy()

        in_maps.append({
            "xb": np.asarray(xb),
            "idx": idx_arr,
            "dstc": dst_arr,
            "xt": xt,
            "bc": bc,
            "w1": np.asarray(conv_w1, np.float32),
            "b1": np.asarray(conv_b1, np.float32).reshape(HID, 1),
            "w2": np.asarray(conv_w2, np.float32),
            "b2": np.asarray(conv_b2, np.float32).reshape(HID, 1),
            "l1w": np.asarray(lin1_w, np.float32),
            "l1b": np.asarray(lin1_b, np.float32).reshape(HID, 1),
            "l2w": np.asarray(lin2_w, np.float32),
            "l2b": np.tile(np.asarray(lin2_b, np.float32), (gpg, 1)),
        })
    return in_maps, NT, G_tb, Np, gpg, core_graphs


_trace = {"on": False, "last": None}


def _pick_blocks(src, core, tt, NT, n_nodes, nblk=4):
    """Choose src-block sizes minimizing total padded groups."""
    best, best_tot = None, None
    for s3 in range(25000, 27100, 250):
        rem = n_nodes - (nblk - 1) * s3
        if rem <= 0 or rem > 32704 or s3 > 32704:
            continue
        blocks = [s3] * (nblk - 1) + [rem + 352]
        bstart = np.concatenate([[0], np.cumsum(blocks)])
        bb = np.searchsorted(bstart, src, side="right") - 1
        key = (core * NT + tt) * nblk + bb
        cnt = np.bincount(key, minlength=NCORES * NT * nblk)
        cnt = cnt.reshape(NCORES, NT, nblk)
        g = np.ceil(cnt.max(axis=0) / P).astype(np.int64)
        g[:, 0] = np.maximum(g[:, 0], 1)
        tot = int(g.sum())
        if best_tot is None or tot < best_tot:
            best, best_tot = blocks, tot
    return best


def kernel(x, edge_index, batch, conv_w1, conv_b1, conv_w2, conv_b2,
           lin1_w, lin1_b, lin2_w, lin2_b):
    n_graphs = 512
    # Uniform blocks: _pick_blocks' uneven split saves ~4% groups in the cost
    # model but measured slower on HW; uniform 25088 is the validated config.
    blocks = [25088] * 4
    in_maps, NT, G_tb, Np, gpg, core_graphs = prep_inputs(
        x, edge_index, batch, conv_w1, conv_b1, conv_w2, conv_b2,
        lin1_w, lin1_b, lin2_w, lin2_b, n_graphs, blocks, 4)
    nc = build_program(NT, G_tb, Np, blocks, gpg)
    res = run_bass_kernel_spmd(nc, in_maps, list(range(NCORES)),
                               trace=_trace["on"])
    _trace["last"] = res
    out = np.zeros((n_graphs, NCLS), np.float32)
    for c in range(NCORES):
        out[core_graphs[c]] = np.asarray(res.results[c]["out"], np.float32)
    return out



# revision 34
# speedup vs baseline: 1.0799x; 1.0799x over previous
"""GIN-style GNN message passing on 8 TRN2 NeuronCores.

Pipeline (per core, nodes sharded by graph id so pooling is local):
  phase 1: edge aggregation  agg[dst] += x[src]
      - edges bucketed by (dst node-tile t, src block b) on host, padded to
        groups of 128; src rows gathered from HBM via gpsimd dma_gather
        (bf16, 256B rows); segment-sum via one-hot matmul into PSUM,
        accumulated into a feature-major aggT SBUF tile.
  phase 2: h = relu(relu((x+agg) @ w1 + b1) @ w2 + b2), pooled per graph
      via one-hot matmul, then the small MLP head + log_softmax.

The bass program is identical across the 8 cores (SPMD); all data-dependent
structure (bucket sizes) is made uniform by padding to the max over cores.
"""
import numpy as np
import ml_dtypes

import concourse.bacc as bacc
import concourse.tile as tile
from concourse import mybir
from concourse.bass_utils import run_bass_kernel_spmd
from concourse.library_config import mlp as mlp_lib

P = 128
F = 128
HID = 128
NCLS = 10
NCORES = 8
CALL_G = 8  # groups per dma_gather call; 1024 descs = HW ring cap (hard)
RING_BYTES = 16384  # dynamic_dma_scratch_size (runtime ignores larger)
GBUF_BUFS = 8

FP32 = mybir.dt.float32
BF16 = mybir.dt.bfloat16
I16 = mybir.dt.int16


def pack_order(sizes):
    """Order buckets so call boundaries (mod CALL_G) hit bucket ends often.

    Greedy residue matching: prefer a bucket whose group count completes
    the current call exactly; else draw from the largest residue class.
    """
    from collections import defaultdict
    rem = defaultdict(list)
    for t, g in enumerate(sizes):
        rem[int(g) % CALL_G].append(t)
    order = []
    f = 0
    for _ in range(len(sizes)):
        need = (-f) % CALL_G
        if rem.get(need):
            cls = need
        else:
            cls = max((k for k in rem if rem[k]), key=lambda k: len(rem[k]))
        t = rem[cls].pop()
        order.append(t)
        f = (f + int(sizes[t])) % CALL_G
    return order


def build_program(NT, G_tb, Np, blocks, GPG, order_tb, rep=1):
    """Build the SPMD bass program.

    NT: node tiles per core; G_tb: [NT, NBLK] groups per bucket; Np: NT*P;
    blocks: src block sizes (each <= 32767 rows); GPG: graphs per core;
    order_tb: per-pass tile emission order (must match host slot layout).
    """
    NBLK = len(blocks)
    bstart = [0]
    for bs in blocks:
        bstart.append(bstart[-1] + bs)
    TOT_G = int(G_tb.sum())
    nc = bacc.Bacc("TRN2", target_bir_lowering=False, debug=False,
                   num_swdge_queues=4, dynamic_dma_scratch_size=RING_BYTES)

    xb_t = nc.declare_dram_parameter("xb", [bstart[-1], F], BF16, isOutput=False)
    idx_t = nc.declare_dram_parameter("idx", [P, TOT_G * 8], I16, isOutput=False)
    dst_t = nc.declare_dram_parameter("dstc", [P, TOT_G], BF16, isOutput=False)
    xt_t = nc.declare_dram_parameter("xt", [P, Np], FP32, isOutput=False)
    bc_t = nc.declare_dram_parameter("bc", [P, NT], FP32, isOutput=False)
    w1_t = nc.declare_dram_parameter("w1", [F, HID], FP32, isOutput=False)
    b1_t = nc.declare_dram_parameter("b1", [HID, 1], FP32, isOutput=False)
    w2_t = nc.declare_dram_parameter("w2", [HID, HID], FP32, isOutput=False)
    b2_t = nc.declare_dram_parameter("b2", [HID, 1], FP32, isOutput=False)
    l1w_t = nc.declare_dram_parameter("l1w", [HID, HID], FP32, isOutput=False)
    l1b_t = nc.declare_dram_parameter("l1b", [HID, 1], FP32, isOutput=False)
    l2w_t = nc.declare_dram_parameter("l2w", [HID, NCLS], FP32, isOutput=False)
    l2b_t = nc.declare_dram_parameter("l2b", [GPG, NCLS], FP32, isOutput=False)
    out_t = nc.declare_dram_parameter("out", [GPG, NCLS], FP32, isOutput=True)

    import ml_dtypes as _mld
    iota_c = nc.inline_tensor(
        np.tile(np.arange(P, dtype=_mld.bfloat16), (P, CALL_G)), name="iota128")
    iotag_c = nc.inline_tensor(
        np.tile(np.arange(GPG, dtype=np.float32), (P, 4)), name="iotag")
    ident_c = nc.inline_tensor(np.eye(P, dtype=np.float32), name="ident")

    with tile.TileContext(nc) as tc:
        nc.gpsimd.load_library(mlp_lib)
        with tc.tile_pool(name="const", bufs=1) as cpool, \
             tc.tile_pool(name="agg", bufs=NT) as apool, \
             tc.tile_pool(name="gbuf", bufs=GBUF_BUFS) as gpool, \
             tc.tile_pool(name="ibuf", bufs=2) as ipool, \
             tc.tile_pool(name="oh", bufs=4) as ohpool, \
             tc.tile_pool(name="p2s", bufs=6) as spool, \
             tc.tile_pool(name="psum1", bufs=3, space="PSUM") as p1, \
             tc.tile_pool(name="psum2", bufs=2, space="PSUM") as p2, \
             tc.tile_pool(name="psumg", bufs=1, space="PSUM") as pg:

            iota_sb = cpool.tile([P, CALL_G * P], BF16)
            nc.sync.dma_start(out=iota_sb[:], in_=iota_c[:])
            iotag_sb = cpool.tile([P, 4 * GPG], FP32)
            nc.sync.dma_start(out=iotag_sb[:], in_=iotag_c[:])
            ident_sb = cpool.tile([P, P], FP32)
            nc.sync.dma_start(out=ident_sb[:], in_=ident_c[:])
            dstc_sb = cpool.tile([P, TOT_G], BF16)
            nc.sync.dma_start(out=dstc_sb[:], in_=dst_t[:])
            bc_sb = cpool.tile([P, NT], FP32)
            nc.sync.dma_start(out=bc_sb[:], in_=bc_t[:])
            w1_sb = cpool.tile([F, HID], FP32)
            nc.sync.dma_start(out=w1_sb[:], in_=w1_t[:])
            b1_sb = cpool.tile([HID, 1], FP32)
            nc.sync.dma_start(out=b1_sb[:], in_=b1_t[:])
            w2_sb = cpool.tile([HID, HID], FP32)
            nc.sync.dma_start(out=w2_sb[:], in_=w2_t[:])
            b2_sb = cpool.tile([HID, 1], FP32)
            nc.sync.dma_start(out=b2_sb[:], in_=b2_t[:])
            l1w_sb = cpool.tile([HID, HID], FP32)
            nc.sync.dma_start(out=l1w_sb[:], in_=l1w_t[:])
            l1b_sb = cpool.tile([HID, 1], FP32)
            nc.sync.dma_start(out=l1b_sb[:], in_=l1b_t[:])
            l2w_sb = cpool.tile([HID, NCLS], FP32)
            nc.sync.dma_start(out=l2w_sb[:], in_=l2w_t[:])
            l2b_sb = cpool.tile([GPG, NCLS], FP32)
            nc.sync.dma_start(out=l2b_sb[:], in_=l2b_t[:])

            # repeated body (rep>1 used only for benchmarking)
            for _rep in range(rep):
                aggts = []
                for _t in range(NT):
                    agg_tile = apool.tile([P, P], FP32, tag="aggt")
                    aggts.append(agg_tile)

                # ---- phase 1 + interleaved phase 2 ----
                gacc = pg.tile([GPG, HID], FP32)
                CH = 4  # phase-2 tiles per chunk; rhs width CH*P = 512

                def emit_phase2(c):
                    """MLP + pooling for node tiles [c*CH, min(NT,(c+1)*CH))."""
                    t0 = c * CH
                    w = min(CH, NT - t0) * P
                    xt_sb = spool.tile([P, CH * P], FP32, tag="xt")
                    nc.sync.dma_start(out=xt_sb[:, :w],
                                      in_=xt_t[:, t0 * P:t0 * P + w])
                    hin = spool.tile([P, CH * P], FP32, tag="hin")
                    for i in range(w // P):
                        nc.vector.tensor_add(
                            out=hin[:, i * P:(i + 1) * P],
                            in0=xt_sb[:, i * P:(i + 1) * P],
                            in1=aggts[t0 + i][:])
                    ps1 = p2.tile([P, CH * P], FP32, tag="wide")
                    nc.tensor.matmul(out=ps1[:, :w], lhsT=w1_sb[:],
                                     rhs=hin[:, :w], start=True, stop=True)
                    h1 = spool.tile([P, CH * P], FP32, tag="h1")
                    nc.scalar.activation(out=h1[:, :w], in_=ps1[:, :w],
                                         func=mybir.ActivationFunctionType.Relu,
                                         bias=b1_sb[:, 0:1])
                    ps2 = p2.tile([P, CH * P], FP32, tag="wide")
                    nc.tensor.matmul(out=ps2[:, :w], lhsT=w2_sb[:],
                                     rhs=h1[:, :w], start=True, stop=True)
                    h2 = spool.tile([P, CH * P], FP32, tag="h2")
                    nc.scalar.activation(out=h2[:, :w], in_=ps2[:, :w],
                                         func=mybir.ActivationFunctionType.Relu,
                                         bias=b2_sb[:, 0:1])
                    nch = w // P
                    ohg = ohpool.tile([P, 4, GPG], FP32, tag="ohg")
                    nc.vector.tensor_tensor(
                        out=ohg[:, :nch, :], in0=iotag_sb[:, :nch * GPG],
                        in1=bc_sb[:, t0:t0 + nch].unsqueeze(2)
                            .broadcast_to([P, nch, GPG]),
                        op=mybir.AluOpType.is_equal)
                    for i in range(nch):
                        t = t0 + i
                        ps3 = p2.tile([P, P], FP32, tag="ps")
                        nc.tensor.transpose(out=ps3[:],
                                            in_=h2[:, i * P:(i + 1) * P],
                                            identity=ident_sb[:])
                        h2t = spool.tile([P, P], FP32, tag="h2t")
                        nc.vector.tensor_copy(out=h2t[:], in_=ps3[:])
                        nc.tensor.matmul(out=gacc[:], lhsT=ohg[:, i, :],
                                         rhs=h2t[:],
                                         start=(t == 0), stop=(t == NT - 1))

                max_gb_pass = max(int(G_tb[:, b].sum()) for b in range(NBLK))
                ci = 0  # global call counter (queue = ci % 4)
                goff = 0  # global group index (b-major order, matches host layout)
                p2_next = 0  # next phase-2 chunk to emit (during last pass)
                for b in range(NBLK):
                    seq = []  # (t, j, glast) for each group of this pass
                    for t in order_tb[b]:
                        gb = int(G_tb[t, b])
                        for j in range(gb):
                            seq.append((t, j, gb))
                    nb = len(seq)
                    idx_sb = ipool.tile([P, max_gb_pass * 8], I16, tag="idx")
                    nc.sync.dma_start(
                        out=idx_sb[:, :nb * 8],
                        in_=idx_t[:, goff * 8:(goff + nb) * 8])
                    # chunk into gather calls
                    for c0 in range(0, nb, CALL_G):
                        ng = min(CALL_G, nb - c0)
                        q = ci % 4
                        ci += 1
                        gbuf = gpool.tile([P, CALL_G, F], BF16, tag="gbuf")
                        if ci <= GBUF_BUFS:
                            # first use of each rotating buffer: clear stale
                            # lanes (pad slots trimmed from the gather leave
                            # whatever SBUF held; 0 * 0-onehot stays finite)
                            nc.vector.memset(gbuf[:], 0.0)
                        nc.gpsimd.dma_gather(
                            out_ap=gbuf[:, :ng, :],
                            in_ap=xb_t[bstart[b]:bstart[b + 1], :],
                            idxs_ap=idx_sb[:, c0 * 8:(c0 + ng) * 8],
                            num_idxs=ng * P,
                            num_idxs_reg=ng * P,
                            elem_size=F,
                            queue_num=q,
                        )
                        seq_c = seq[c0:c0 + ng]
                        oh = ohpool.tile([P, CALL_G, P], BF16, tag="oh")
                        g0 = goff + c0
                        nc.vector.tensor_tensor(
                            out=oh[:, :ng, :], in0=iota_sb[:, :ng * P],
                            in1=dstc_sb[:, g0:g0 + ng].unsqueeze(2)
                                .broadcast_to([P, ng, P]),
                            op=mybir.AluOpType.is_equal)
                        for s, (t, j, gb) in enumerate(seq_c):
                            if j == 0:
                                ps = p1.tile([P, P], FP32, tag="p1")
                                cur_ps = ps
                            else:
                                ps = cur_ps
                            nc.tensor.matmul(
                                out=ps[:], lhsT=gbuf[:, s, :], rhs=oh[:, s, :],
                                start=(j == 0), stop=(j == gb - 1))
                            if j == gb - 1:
                                sl = aggts[t][:]
                                if b == 0:
                                    nc.scalar.copy(out=sl, in_=ps[:])
                                else:
                                    nc.vector.tensor_add(out=sl, in0=sl, in1=ps[:])
                    goff += nb
                while p2_next * CH < NT:
                    emit_phase2(p2_next)
                    p2_next += 1

                # ---- head MLP + log_softmax ----
                g_sb = spool.tile([GPG, HID], FP32, tag="g")
                nc.scalar.copy(out=g_sb[:], in_=gacc[:])
                psg = p2.tile([HID, GPG], FP32, tag="ps")
                nc.tensor.transpose(out=psg[:], in_=g_sb[:],
                                    identity=ident_sb[:GPG, :GPG])
                gt = spool.tile([HID, GPG], FP32, tag="gt")
                nc.vector.tensor_copy(out=gt[:], in_=psg[:])
                ps4 = p2.tile([HID, GPG], FP32, tag="ps")
                nc.tensor.matmul(out=ps4[:], lhsT=l1w_sb[:], rhs=gt[:],
                                 start=True, stop=True)
                g1 = spool.tile([HID, GPG], FP32, tag="g1")
                nc.scalar.activation(out=g1[:], in_=ps4[:],
                                     func=mybir.ActivationFunctionType.Relu,
                                     bias=l1b_sb[:, 0:1])
                ps5 = p2.tile([GPG, NCLS], FP32, tag="ps")
                nc.tensor.matmul(out=ps5[:], lhsT=g1[:], rhs=l2w_sb[:],
                                 start=True, stop=True)
                logits = spool.tile([GPG, NCLS], FP32, tag="lg")
                nc.vector.tensor_tensor(out=logits[:], in0=ps5[:], in1=l2b_sb[:],
                                        op=mybir.AluOpType.add)
                mx = spool.tile([GPG, 1], FP32, tag="mx")
                nc.vector.tensor_reduce(out=mx[:], in_=logits[:],
                                        axis=mybir.AxisListType.X,
                                        op=mybir.AluOpType.max)
                sh = spool.tile([GPG, NCLS], FP32, tag="sh")
                nc.vector.tensor_scalar(out=sh[:], in0=logits[:],
                                        scalar1=mx[:, 0:1], scalar2=None,
                                        op0=mybir.AluOpType.subtract)
                ex = spool.tile([GPG, NCLS], FP32, tag="ex")
                ssum = spool.tile([GPG, 1], FP32, tag="ssum")
                nc.scalar.activation(out=ex[:], in_=sh[:],
                                     func=mybir.ActivationFunctionType.Exp,
                                     accum_out=ssum[:])
                lse = spool.tile([GPG, 1], FP32, tag="lse")
                nc.scalar.activation(out=lse[:], in_=ssum[:],
                                     func=mybir.ActivationFunctionType.Ln)
                res = spool.tile([GPG, NCLS], FP32, tag="res")
                nc.vector.tensor_scalar(out=res[:], in0=sh[:],
                                        scalar1=lse[:, 0:1], scalar2=None,
                                        op0=mybir.AluOpType.subtract)
                nc.sync.dma_start(out=out_t[:], in_=res[:])

    nc.compile()
    return nc


def assign_graphs(batch, dst, n_graphs, gpg):
    """LPT bin-packing of graphs onto cores by edge count, node-capped.

    Returns core_graphs [NCORES, gpg] (graph ids per core, sorted) such
    that per-core edge totals are balanced and node counts fit min NT.
    """
    gnodes = np.bincount(batch, minlength=n_graphs)
    gedges = np.bincount(batch[dst], minlength=n_graphs)
    node_cap = int(np.ceil(gnodes.sum() / NCORES / P)) * P  # target NT
    order = np.argsort(-gedges, kind="stable")
    ce = np.zeros(NCORES, np.int64)
    cn = np.zeros(NCORES, np.int64)
    cg = np.zeros(NCORES, np.int64)
    out = [[] for _ in range(NCORES)]
    for g in order:
        # least-loaded (edges) core with node+graph capacity
        best, be = -1, None
        for c in range(NCORES):
            if cg[c] < gpg and cn[c] + gnodes[g] <= node_cap:
                if be is None or ce[c] < be:
                    best, be = c, ce[c]
        if best < 0:  # node cap infeasible; fall back to graph-count only
            best = min((c for c in range(NCORES) if cg[c] < gpg),
                       key=lambda c: ce[c])
        out[best].append(g)
        ce[best] += gedges[g]
        cn[best] += gnodes[g]
        cg[best] += 1
    return [np.sort(np.array(gs, np.int64)) for gs in out]


def prep_inputs(x, edge_index, batch, conv_w1, conv_b1, conv_w2, conv_b2,
                lin1_w, lin1_b, lin2_w, lin2_b, n_graphs, blk, nblk):
    """Host-side sharding: returns (in_maps, NT, G_tb, Np, GPG, core_graphs).

    blk: int (uniform block size, nblk blocks) or list of block sizes.
    """
    blocks = [blk] * nblk if isinstance(blk, int) else list(blk)
    nblk = len(blocks)
    bstart = np.concatenate([[0], np.cumsum(blocks)])
    n_nodes = x.shape[0]
    x = np.asarray(x, np.float32)
    batch = np.asarray(batch, np.int64)
    src = np.asarray(edge_index[0], np.int64)
    dst = np.asarray(edge_index[1], np.int64)
    gpg = n_graphs // NCORES

    gbounds = np.concatenate(
        [[0], np.cumsum(np.bincount(batch, minlength=n_graphs))])
    core_graphs = assign_graphs(batch, dst, n_graphs, gpg)

    # per-node (core, local position); graphs keep contiguous node runs
    node_core = np.empty(n_nodes, np.int32)
    node_local = np.empty(n_nodes, np.int64)
    core_nodes = []  # global node ids per core, in local order
    counts = np.zeros(NCORES, np.int64)
    for c in range(NCORES):
        ids = np.concatenate([np.arange(gbounds[g], gbounds[g + 1])
                              for g in core_graphs[c]])
        core_nodes.append(ids)
        node_core[ids] = c
        node_local[ids] = np.arange(len(ids))
        counts[c] = len(ids)
    NT = max(1, int(np.ceil(counts.max() / P)))
    Np = NT * P

    core = node_core[dst].astype(np.int64)
    nlocal = node_local[dst]
    tt = nlocal // P
    dl = nlocal % P
    bb = np.searchsorted(bstart, src, side="right") - 1
    sl = src - bstart[bb]

    key = (core * NT + tt) * nblk + bb
    cnt = np.bincount(key, minlength=NCORES * NT * nblk).reshape(NCORES, NT, nblk)
    G_tb = np.ceil(cnt.max(axis=0) / P).astype(np.int64)  # [NT, nblk]
    G_tb[:, 0] = np.maximum(G_tb[:, 0], 1)
    TOT_G = int(G_tb.sum())

    # padded slot layout, b-major then pack_order-of-t (must match device
    # emission): call boundaries align with bucket ends so per-core pad
    # slots can be -1-trimmed from the gather at runtime.
    order_tb = [pack_order(G_tb[:, b]) for b in range(nblk)]
    slot_off = np.zeros((nblk, NT), np.int64)
    pass_start = []
    off = 0
    for b in range(nblk):
        pass_start.append(off)
        for t in order_tb[b]:
            slot_off[b, t] = off
            off += int(G_tb[t, b]) * P
    total_slots = TOT_G * P
    assert off == total_slots

    # bf16 x table, padded rows
    xpad = np.zeros((int(bstart[-1]), F), np.float32)
    xpad[:n_nodes] = x
    xb = xpad.astype(ml_dtypes.bfloat16)

    in_maps = []
    for c in range(NCORES):
        m = core == c
        sl_c, dl_c, tt_c, bb_c = sl[m], dl[m], tt[m], bb[m]
        order = np.lexsort((sl_c, tt_c, bb_c))
        sl_c, dl_c, tt_c, bb_c = (sl_c[order], dl_c[order], tt_c[order],
                                  bb_c[order])
        # rank within bucket (edges sorted by (b, t); buckets contiguous)
        bucket = bb_c * NT + tt_c
        changes = np.concatenate([[True], bucket[1:] != bucket[:-1]])
        idx_in_run = np.arange(len(bucket)) - \
            np.maximum.accumulate(np.where(changes, np.arange(len(bucket)), 0))
        pos = slot_off[bb_c, tt_c] + idx_in_run

        SL = np.zeros(total_slots, np.int16)
        DL = np.full(total_slots, 255.0, np.float32)
        SL[pos] = sl_c.astype(np.int16)
        DL[pos] = dl_c.astype(np.float32)

        # -1 on call-trailing pad slots: the gather ucode trims trailing
        # negative idxs at runtime, skipping those descriptors per core.
        real = DL != 255.0
        for b in range(nblk):
            s0 = pass_start[b]
            nslots = int(G_tb[:, b].sum()) * P
            for a in range(s0, s0 + nslots, CALL_G * P):
                e = min(a + CALL_G * P, s0 + nslots)
                r = np.nonzero(real[a:e])[0]
                cut = a + (int(r[-1]) + 1 if len(r) else 0)
                SL[cut:e] = -1

        idx_arr = np.tile(SL.reshape(-1, 16).T, (8, 1)).astype(np.int16)
        dst_arr = DL.reshape(TOT_G, P).T.astype(ml_dtypes.bfloat16)

        ids, cn = core_nodes[c], counts[c]
        xt = np.zeros((P, Np), np.float32)
        xt[:, :cn] = x[ids].T
        g2l = np.full(n_graphs, 255, np.int64)
        g2l[core_graphs[c]] = np.arange(gpg)
        bc = np.full(Np, 255.0, np.float32)
        bc[:cn] = g2l[batch[ids]].astype(np.float32)
        bc = bc.reshape(NT, P).T.copy()

        in_maps.append({
            "xb": np.asarray(xb),
            "idx": idx_arr,
            "dstc": dst_arr,
            "xt": xt,
            "bc": bc,
            "w1": np.asarray(conv_w1, np.float32),
            "b1": np.asarray(conv_b1, np.float32).reshape(HID, 1),
            "w2": np.asarray(conv_w2, np.float32),
            "b2": np.asarray(conv_b2, np.float32).reshape(HID, 1),
            "l1w": np.asarray(lin1_w, np.float32),
            "l1b": np.asarray(lin1_b, np.float32).reshape(HID, 1),
            "l2w": np.asarray(lin2_w, np.float32),
            "l2b": np.tile(np.asarray(lin2_b, np.float32), (gpg, 1)),
        })
    return in_maps, NT, G_tb, Np, gpg, core_graphs, order_tb


_trace = {"on": False, "last": None}


def _pick_blocks(src, core, tt, NT, n_nodes, nblk=4):
    """Choose src-block sizes minimizing total padded groups."""
    best, best_tot = None, None
    for s3 in range(25000, 27100, 250):
        rem = n_nodes - (nblk - 1) * s3
        if rem <= 0 or rem > 32704 or s3 > 32704:
            continue
        blocks = [s3] * (nblk - 1) + [rem + 352]
        bstart = np.concatenate([[0], np.cumsum(blocks)])
        bb = np.searchsorted(bstart, src, side="right") - 1
        key = (core * NT + tt) * nblk + bb
        cnt = np.bincount(key, minlength=NCORES * NT * nblk)
        cnt = cnt.reshape(NCORES, NT, nblk)
        g = np.ceil(cnt.max(axis=0) / P).astype(np.int64)
        g[:, 0] = np.maximum(g[:, 0], 1)
        tot = int(g.sum())
        if best_tot is None or tot < best_tot:
            best, best_tot = blocks, tot
    return best


def kernel(x, edge_index, batch, conv_w1, conv_b1, conv_w2, conv_b2,
           lin1_w, lin1_b, lin2_w, lin2_b):
    n_graphs = 512
    # Uniform blocks: _pick_blocks' uneven split saves ~4% groups in the cost
    # model but measured slower on HW; uniform 25088 is the validated config.
    blocks = [25088] * 4
    in_maps, NT, G_tb, Np, gpg, core_graphs, order_tb = prep_inputs(
        x, edge_index, batch, conv_w1, conv_b1, conv_w2, conv_b2,
        lin1_w, lin1_b, lin2_w, lin2_b, n_graphs, blocks, 4)
    nc = build_program(NT, G_tb, Np, blocks, gpg, order_tb)
    res = run_bass_kernel_spmd(nc, in_maps, list(range(NCORES)),
                               trace=_trace["on"])
    _trace["last"] = res
    out = np.zeros((n_graphs, NCLS), np.float32)
    for c in range(NCORES):
        out[core_graphs[c]] = np.asarray(res.results[c]["out"], np.float32)
    return out



# revision 37
# speedup vs baseline: 1.0896x; 1.0090x over previous
"""GIN-style GNN message passing on 8 TRN2 NeuronCores.

Pipeline (per core, nodes sharded by graph id so pooling is local):
  phase 1: edge aggregation  agg[dst] += x[src]
      - edges bucketed by (dst node-tile t, src block b) on host, padded to
        groups of 128; src rows gathered from HBM via gpsimd dma_gather
        (bf16, 256B rows); segment-sum via one-hot matmul into PSUM,
        accumulated into a feature-major aggT SBUF tile.
  phase 2: h = relu(relu((x+agg) @ w1 + b1) @ w2 + b2), pooled per graph
      via one-hot matmul, then the small MLP head + log_softmax.

The bass program is identical across the 8 cores (SPMD); all data-dependent
structure (bucket sizes) is made uniform by padding to the max over cores.
"""
import numpy as np
import ml_dtypes

import concourse.bacc as bacc
import concourse.tile as tile
from concourse import mybir
from concourse.bass_utils import run_bass_kernel_spmd
from concourse.library_config import mlp as mlp_lib

P = 128
F = 128
HID = 128
NCLS = 10
NCORES = 8
CALL_G = 8  # groups per dma_gather call; 1024 descs = HW ring cap (hard)
RING_BYTES = 16384  # dynamic_dma_scratch_size (runtime ignores larger)
GBUF_BUFS = 8

FP32 = mybir.dt.float32
BF16 = mybir.dt.bfloat16
I16 = mybir.dt.int16


def pack_order(sizes):
    """Order buckets so call boundaries (mod CALL_G) hit bucket ends often.

    Greedy residue matching: prefer a bucket whose group count completes
    the current call exactly; else draw from the largest residue class.
    """
    from collections import defaultdict
    rem = defaultdict(list)
    for t, g in enumerate(sizes):
        rem[int(g) % CALL_G].append(t)
    order = []
    f = 0
    for _ in range(len(sizes)):
        need = (-f) % CALL_G
        if rem.get(need):
            cls = need
        else:
            cls = max((k for k in rem if rem[k]), key=lambda k: len(rem[k]))
        t = rem[cls].pop()
        order.append(t)
        f = (f + int(sizes[t])) % CALL_G
    return order


def build_program(NT, G_tb, Np, blocks, GPG, order_tb, rep=1):
    """Build the SPMD bass program.

    NT: node tiles per core; G_tb: [NT, NBLK] groups per bucket; Np: NT*P;
    blocks: src block sizes (each <= 32767 rows); GPG: graphs per core;
    order_tb: per-pass tile emission order (must match host slot layout).
    """
    NBLK = len(blocks)
    bstart = [0]
    for bs in blocks:
        bstart.append(bstart[-1] + bs)
    TOT_G = int(G_tb.sum())
    nc = bacc.Bacc("TRN2", target_bir_lowering=False, debug=False,
                   num_swdge_queues=4, dynamic_dma_scratch_size=RING_BYTES)

    xb_t = nc.declare_dram_parameter("xb", [bstart[-1], F], BF16, isOutput=False)
    idx_t = nc.declare_dram_parameter("idx", [P, TOT_G * 8], I16, isOutput=False)
    dst_t = nc.declare_dram_parameter("dstc", [P, TOT_G], BF16, isOutput=False)
    xt_t = nc.declare_dram_parameter("xt", [P, Np], FP32, isOutput=False)
    bc_t = nc.declare_dram_parameter("bc", [P, NT], FP32, isOutput=False)
    w1_t = nc.declare_dram_parameter("w1", [F, HID], FP32, isOutput=False)
    b1_t = nc.declare_dram_parameter("b1", [HID, 1], FP32, isOutput=False)
    w2_t = nc.declare_dram_parameter("w2", [HID, HID], FP32, isOutput=False)
    b2_t = nc.declare_dram_parameter("b2", [HID, 1], FP32, isOutput=False)
    l1w_t = nc.declare_dram_parameter("l1w", [HID, HID], FP32, isOutput=False)
    l1b_t = nc.declare_dram_parameter("l1b", [HID, 1], FP32, isOutput=False)
    l2w_t = nc.declare_dram_parameter("l2w", [HID, NCLS], FP32, isOutput=False)
    l2b_t = nc.declare_dram_parameter("l2b", [GPG, NCLS], FP32, isOutput=False)
    out_t = nc.declare_dram_parameter("out", [GPG, NCLS], FP32, isOutput=True)

    import ml_dtypes as _mld
    iota_c = nc.inline_tensor(
        np.tile(np.arange(P, dtype=_mld.bfloat16), (P, CALL_G)), name="iota128")
    iotag_c = nc.inline_tensor(
        np.tile(np.arange(GPG, dtype=np.float32), (P, 4)), name="iotag")
    ident_c = nc.inline_tensor(np.eye(P, dtype=np.float32), name="ident")

    with tile.TileContext(nc) as tc:
        nc.gpsimd.load_library(mlp_lib)
        with tc.tile_pool(name="const", bufs=1) as cpool, \
             tc.tile_pool(name="agg", bufs=NT) as apool, \
             tc.tile_pool(name="gbuf", bufs=GBUF_BUFS) as gpool, \
             tc.tile_pool(name="ibuf", bufs=2) as ipool, \
             tc.tile_pool(name="oh", bufs=4) as ohpool, \
             tc.tile_pool(name="p2s", bufs=6) as spool, \
             tc.tile_pool(name="psum1", bufs=3, space="PSUM") as p1, \
             tc.tile_pool(name="psum2", bufs=2, space="PSUM") as p2, \
             tc.tile_pool(name="psumt", bufs=2, space="PSUM") as pt, \
             tc.tile_pool(name="psumg", bufs=1, space="PSUM") as pg:

            iota_sb = cpool.tile([P, CALL_G * P], BF16)
            nc.sync.dma_start(out=iota_sb[:], in_=iota_c[:])
            iotag_sb = cpool.tile([P, 4 * GPG], FP32)
            nc.sync.dma_start(out=iotag_sb[:], in_=iotag_c[:])
            ident_sb = cpool.tile([P, P], FP32)
            nc.sync.dma_start(out=ident_sb[:], in_=ident_c[:])
            dstc_sb = cpool.tile([P, TOT_G], BF16)
            nc.sync.dma_start(out=dstc_sb[:], in_=dst_t[:])
            bc_sb = cpool.tile([P, NT], FP32)
            nc.sync.dma_start(out=bc_sb[:], in_=bc_t[:])
            w1_sb = cpool.tile([F, HID], FP32)
            nc.sync.dma_start(out=w1_sb[:], in_=w1_t[:])
            b1_sb = cpool.tile([HID, 1], FP32)
            nc.sync.dma_start(out=b1_sb[:], in_=b1_t[:])
            w2_sb = cpool.tile([HID, HID], FP32)
            nc.sync.dma_start(out=w2_sb[:], in_=w2_t[:])
            b2_sb = cpool.tile([HID, 1], FP32)
            nc.sync.dma_start(out=b2_sb[:], in_=b2_t[:])
            l1w_sb = cpool.tile([HID, HID], FP32)
            nc.sync.dma_start(out=l1w_sb[:], in_=l1w_t[:])
            l1b_sb = cpool.tile([HID, 1], FP32)
            nc.sync.dma_start(out=l1b_sb[:], in_=l1b_t[:])
            l2w_sb = cpool.tile([HID, NCLS], FP32)
            nc.sync.dma_start(out=l2w_sb[:], in_=l2w_t[:])
            l2b_sb = cpool.tile([GPG, NCLS], FP32)
            nc.sync.dma_start(out=l2b_sb[:], in_=l2b_t[:])

            # repeated body (rep>1 used only for benchmarking)
            for _rep in range(rep):
                aggts = []
                for _t in range(NT):
                    agg_tile = apool.tile([P, P], FP32, tag="aggt")
                    aggts.append(agg_tile)

                # ---- phase 1 + interleaved phase 2 ----
                gacc = pg.tile([GPG, HID], FP32)
                CH = 4  # phase-2 tiles per chunk; rhs width CH*P = 512

                def emit_phase2(c):
                    """MLP + pooling for node tiles [c*CH, min(NT,(c+1)*CH))."""
                    t0 = c * CH
                    w = min(CH, NT - t0) * P
                    xt_sb = spool.tile([P, CH * P], FP32, tag="xt")
                    nc.sync.dma_start(out=xt_sb[:, :w],
                                      in_=xt_t[:, t0 * P:t0 * P + w])
                    hin = spool.tile([P, CH * P], FP32, tag="hin")
                    for i in range(w // P):
                        nc.vector.tensor_add(
                            out=hin[:, i * P:(i + 1) * P],
                            in0=xt_sb[:, i * P:(i + 1) * P],
                            in1=aggts[t0 + i][:])
                    ps1 = p2.tile([P, CH * P], FP32, tag="wide")
                    nc.tensor.matmul(out=ps1[:, :w], lhsT=w1_sb[:],
                                     rhs=hin[:, :w], start=True, stop=True)
                    h1 = spool.tile([P, CH * P], FP32, tag="h1")
                    nc.scalar.activation(out=h1[:, :w], in_=ps1[:, :w],
                                         func=mybir.ActivationFunctionType.Relu,
                                         bias=b1_sb[:, 0:1])
                    ps2 = p2.tile([P, CH * P], FP32, tag="wide")
                    nc.tensor.matmul(out=ps2[:, :w], lhsT=w2_sb[:],
                                     rhs=h1[:, :w], start=True, stop=True)
                    h2 = spool.tile([P, CH * P], FP32, tag="h2")
                    nc.scalar.activation(out=h2[:, :w], in_=ps2[:, :w],
                                         func=mybir.ActivationFunctionType.Relu,
                                         bias=b2_sb[:, 0:1])
                    nch = w // P
                    ohg = ohpool.tile([P, 4, GPG], FP32, tag="ohg")
                    nc.vector.tensor_tensor(
                        out=ohg[:, :nch, :], in0=iotag_sb[:, :nch * GPG],
                        in1=bc_sb[:, t0:t0 + nch].unsqueeze(2)
                            .broadcast_to([P, nch, GPG]),
                        op=mybir.AluOpType.is_equal)
                    for i in range(nch):
                        t = t0 + i
                        ps3 = pt.tile([P, P], FP32, tag="ps")
                        nc.tensor.transpose(out=ps3[:],
                                            in_=h2[:, i * P:(i + 1) * P],
                                            identity=ident_sb[:])
                        h2t = spool.tile([P, P], FP32, tag="h2t")
                        nc.vector.tensor_copy(out=h2t[:], in_=ps3[:])
                        nc.tensor.matmul(out=gacc[:], lhsT=ohg[:, i, :],
                                         rhs=h2t[:],
                                         start=(t == 0), stop=(t == NT - 1))

                max_gb_pass = max(int(G_tb[:, b].sum()) for b in range(NBLK))
                ci = 0  # global call counter (queue = ci % 4)
                goff = 0  # global group index (b-major order, matches host layout)
                p2_next = 0  # next phase-2 chunk to emit (during last pass)
                for b in range(NBLK):
                    seq = []  # (t, j, glast) for each group of this pass
                    for t in order_tb[b]:
                        gb = int(G_tb[t, b])
                        for j in range(gb):
                            seq.append((t, j, gb))
                    nb = len(seq)
                    idx_sb = ipool.tile([P, max_gb_pass * 8], I16, tag="idx")
                    nc.sync.dma_start(
                        out=idx_sb[:, :nb * 8],
                        in_=idx_t[:, goff * 8:(goff + nb) * 8])
                    # chunk into gather calls
                    for c0 in range(0, nb, CALL_G):
                        ng = min(CALL_G, nb - c0)
                        q = ci % 4
                        ci += 1
                        gbuf = gpool.tile([P, CALL_G, F], BF16, tag="gbuf")
                        nc.gpsimd.dma_gather(
                            out_ap=gbuf[:, :ng, :],
                            in_ap=xb_t[bstart[b]:bstart[b + 1], :],
                            idxs_ap=idx_sb[:, c0 * 8:(c0 + ng) * 8],
                            num_idxs=ng * P,
                            num_idxs_reg=ng * P,
                            elem_size=F,
                            queue_num=q,
                        )
                        seq_c = seq[c0:c0 + ng]
                        oh = ohpool.tile([P, CALL_G, P], BF16, tag="oh")
                        g0 = goff + c0
                        nc.vector.tensor_tensor(
                            out=oh[:, :ng, :], in0=iota_sb[:, :ng * P],
                            in1=dstc_sb[:, g0:g0 + ng].unsqueeze(2)
                                .broadcast_to([P, ng, P]),
                            op=mybir.AluOpType.is_equal)
                        for s, (t, j, gb) in enumerate(seq_c):
                            if j == 0:
                                ps = p1.tile([P, P], FP32, tag="p1")
                                cur_ps = ps
                            else:
                                ps = cur_ps
                            nc.tensor.matmul(
                                out=ps[:], lhsT=gbuf[:, s, :], rhs=oh[:, s, :],
                                start=(j == 0), stop=(j == gb - 1))
                            if j == gb - 1:
                                sl = aggts[t][:]
                                if b == 0:
                                    nc.scalar.copy(out=sl, in_=ps[:])
                                else:
                                    nc.vector.tensor_add(out=sl, in0=sl, in1=ps[:])
                    goff += nb
                while p2_next * CH < NT:
                    emit_phase2(p2_next)
                    p2_next += 1

                # ---- head MLP + log_softmax ----
                g_sb = spool.tile([GPG, HID], FP32, tag="g")
                nc.scalar.copy(out=g_sb[:], in_=gacc[:])
                psg = pt.tile([HID, GPG], FP32, tag="ps")
                nc.tensor.transpose(out=psg[:], in_=g_sb[:],
                                    identity=ident_sb[:GPG, :GPG])
                gt = spool.tile([HID, GPG], FP32, tag="gt")
                nc.vector.tensor_copy(out=gt[:], in_=psg[:])
                ps4 = pt.tile([HID, GPG], FP32, tag="ps")
                nc.tensor.matmul(out=ps4[:], lhsT=l1w_sb[:], rhs=gt[:],
                                 start=True, stop=True)
                g1 = spool.tile([HID, GPG], FP32, tag="g1")
                nc.scalar.activation(out=g1[:], in_=ps4[:],
                                     func=mybir.ActivationFunctionType.Relu,
                                     bias=l1b_sb[:, 0:1])
                ps5 = pt.tile([GPG, NCLS], FP32, tag="ps")
                nc.tensor.matmul(out=ps5[:], lhsT=g1[:], rhs=l2w_sb[:],
                                 start=True, stop=True)
                logits = spool.tile([GPG, NCLS], FP32, tag="lg")
                nc.vector.tensor_tensor(out=logits[:], in0=ps5[:], in1=l2b_sb[:],
                                        op=mybir.AluOpType.add)
                mx = spool.tile([GPG, 1], FP32, tag="mx")
                nc.vector.tensor_reduce(out=mx[:], in_=logits[:],
                                        axis=mybir.AxisListType.X,
                                        op=mybir.AluOpType.max)
                sh = spool.tile([GPG, NCLS], FP32, tag="sh")
                nc.vector.tensor_scalar(out=sh[:], in0=logits[:],
                                        scalar1=mx[:, 0:1], scalar2=None,
                                        op0=mybir.AluOpType.subtract)
                ex = spool.tile([GPG, NCLS], FP32, tag="ex")
                ssum = spool.tile([GPG, 1], FP32, tag="ssum")
                nc.scalar.activation(out=ex[:], in_=sh[:],
                                     func=mybir.ActivationFunctionType.Exp,
                                     accum_out=ssum[:])
                lse = spool.tile([GPG, 1], FP32, tag="lse")
                nc.scalar.activation(out=lse[:], in_=ssum[:],
                                     func=mybir.ActivationFunctionType.Ln)
                res = spool.tile([GPG, NCLS], FP32, tag="res")
                nc.vector.tensor_scalar(out=res[:], in0=sh[:],
                                        scalar1=lse[:, 0:1], scalar2=None,
                                        op0=mybir.AluOpType.subtract)
                nc.sync.dma_start(out=out_t[:], in_=res[:])

    nc.compile()
    return nc


def assign_graphs(batch, dst, n_graphs, gpg):
    """LPT bin-packing of graphs onto cores by edge count, node-capped.

    Returns core_graphs [NCORES, gpg] (graph ids per core, sorted) such
    that per-core edge totals are balanced and node counts fit min NT.
    """
    gnodes = np.bincount(batch, minlength=n_graphs)
    gedges = np.bincount(batch[dst], minlength=n_graphs)
    node_cap = int(np.ceil(gnodes.sum() / NCORES / P)) * P  # target NT
    order = np.argsort(-gedges, kind="stable")
    ce = np.zeros(NCORES, np.int64)
    cn = np.zeros(NCORES, np.int64)
    cg = np.zeros(NCORES, np.int64)
    out = [[] for _ in range(NCORES)]
    for g in order:
        # least-loaded (edges) core with node+graph capacity
        best, be = -1, None
        for c in range(NCORES):
            if cg[c] < gpg and cn[c] + gnodes[g] <= node_cap:
                if be is None or ce[c] < be:
                    best, be = c, ce[c]
        if best < 0:  # node cap infeasible; fall back to graph-count only
            best = min((c for c in range(NCORES) if cg[c] < gpg),
                       key=lambda c: ce[c])
        out[best].append(g)
        ce[best] += gedges[g]
        cn[best] += gnodes[g]
        cg[best] += 1
    return [np.sort(np.array(gs, np.int64)) for gs in out]


def prep_inputs(x, edge_index, batch, conv_w1, conv_b1, conv_w2, conv_b2,
                lin1_w, lin1_b, lin2_w, lin2_b, n_graphs, blk, nblk):
    """Host-side sharding: returns (in_maps, NT, G_tb, Np, GPG, core_graphs).

    blk: int (uniform block size, nblk blocks) or list of block sizes.
    """
    blocks = [blk] * nblk if isinstance(blk, int) else list(blk)
    nblk = len(blocks)
    bstart = np.concatenate([[0], np.cumsum(blocks)])
    n_nodes = x.shape[0]
    x = np.asarray(x, np.float32)
    batch = np.asarray(batch, np.int64)
    src = np.asarray(edge_index[0], np.int64)
    dst = np.asarray(edge_index[1], np.int64)
    gpg = n_graphs // NCORES

    gbounds = np.concatenate(
        [[0], np.cumsum(np.bincount(batch, minlength=n_graphs))])
    core_graphs = assign_graphs(batch, dst, n_graphs, gpg)

    # per-node (core, local position); graphs keep contiguous node runs
    node_core = np.empty(n_nodes, np.int32)
    node_local = np.empty(n_nodes, np.int64)
    core_nodes = []  # global node ids per core, in local order
    counts = np.zeros(NCORES, np.int64)
    for c in range(NCORES):
        ids = np.concatenate([np.arange(gbounds[g], gbounds[g + 1])
                              for g in core_graphs[c]])
        core_nodes.append(ids)
        node_core[ids] = c
        node_local[ids] = np.arange(len(ids))
        counts[c] = len(ids)
    NT = max(1, int(np.ceil(counts.max() / P)))
    Np = NT * P

    core = node_core[dst].astype(np.int64)
    nlocal = node_local[dst]
    tt = nlocal // P
    dl = nlocal % P
    bb = np.searchsorted(bstart, src, side="right") - 1
    sl = src - bstart[bb]

    key = (core * NT + tt) * nblk + bb
    cnt = np.bincount(key, minlength=NCORES * NT * nblk).reshape(NCORES, NT, nblk)
    G_tb = np.ceil(cnt.max(axis=0) / P).astype(np.int64)  # [NT, nblk]
    G_tb[:, 0] = np.maximum(G_tb[:, 0], 1)
    TOT_G = int(G_tb.sum())

    # padded slot layout, b-major then pack_order-of-t (must match device
    # emission): call boundaries align with bucket ends so per-core pad
    # slots can be -1-trimmed from the gather at runtime.
    order_tb = [pack_order(G_tb[:, b]) for b in range(nblk)]
    slot_off = np.zeros((nblk, NT), np.int64)
    pass_start = []
    off = 0
    for b in range(nblk):
        pass_start.append(off)
        for t in order_tb[b]:
            slot_off[b, t] = off
            off += int(G_tb[t, b]) * P
    total_slots = TOT_G * P
    assert off == total_slots

    # bf16 x table, padded rows
    xpad = np.zeros((int(bstart[-1]), F), np.float32)
    xpad[:n_nodes] = x
    xb = xpad.astype(ml_dtypes.bfloat16)

    in_maps = []
    for c in range(NCORES):
        m = core == c
        sl_c, dl_c, tt_c, bb_c = sl[m], dl[m], tt[m], bb[m]
        order = np.lexsort((sl_c, tt_c, bb_c))
        sl_c, dl_c, tt_c, bb_c = (sl_c[order], dl_c[order], tt_c[order],
                                  bb_c[order])
        # rank within bucket (edges sorted by (b, t); buckets contiguous)
        bucket = bb_c * NT + tt_c
        changes = np.concatenate([[True], bucket[1:] != bucket[:-1]])
        idx_in_run = np.arange(len(bucket)) - \
            np.maximum.accumulate(np.where(changes, np.arange(len(bucket)), 0))
        pos = slot_off[bb_c, tt_c] + idx_in_run

        SL = np.zeros(total_slots, np.int16)
        DL = np.full(total_slots, 255.0, np.float32)
        SL[pos] = sl_c.astype(np.int16)
        DL[pos] = dl_c.astype(np.float32)

        idx_arr = np.tile(SL.reshape(-1, 16).T, (8, 1)).astype(np.int16)
        dst_arr = DL.reshape(TOT_G, P).T.astype(ml_dtypes.bfloat16)

        ids, cn = core_nodes[c], counts[c]
        xt = np.zeros((P, Np), np.float32)
        xt[:, :cn] = x[ids].T
        g2l = np.full(n_graphs, 255, np.int64)
        g2l[core_graphs[c]] = np.arange(gpg)
        bc = np.full(Np, 255.0, np.float32)
        bc[:cn] = g2l[batch[ids]].astype(np.float32)
        bc = bc.reshape(NT, P).T.copy()

        in_maps.append({
            "xb": np.asarray(xb),
            "idx": idx_arr,
            "dstc": dst_arr,
            "xt": xt,
            "bc": bc,
            "w1": np.asarray(conv_w1, np.float32),
            "b1": np.asarray(conv_b1, np.float32).reshape(HID, 1),
            "w2": np.asarray(conv_w2, np.float32),
            "b2": np.asarray(conv_b2, np.float32).reshape(HID, 1),
            "l1w": np.asarray(lin1_w, np.float32),
            "l1b": np.asarray(lin1_b, np.float32).reshape(HID, 1),
            "l2w": np.asarray(lin2_w, np.float32),
            "l2b": np.tile(np.asarray(lin2_b, np.float32), (gpg, 1)),
        })
    return in_maps, NT, G_tb, Np, gpg, core_graphs, order_tb


_trace = {"on": False, "last": None}


def _pick_blocks(src, core, tt, NT, n_nodes, nblk=4):
    """Choose src-block sizes minimizing total padded groups."""
    best, best_tot = None, None
    for s3 in range(25000, 27100, 250):
        rem = n_nodes - (nblk - 1) * s3
        if rem <= 0 or rem > 32704 or s3 > 32704:
            continue
        blocks = [s3] * (nblk - 1) + [rem + 352]
        bstart = np.concatenate([[0], np.cumsum(blocks)])
        bb = np.searchsorted(bstart, src, side="right") - 1
        key = (core * NT + tt) * nblk + bb
        cnt = np.bincount(key, minlength=NCORES * NT * nblk)
        cnt = cnt.reshape(NCORES, NT, nblk)
        g = np.ceil(cnt.max(axis=0) / P).astype(np.int64)
        g[:, 0] = np.maximum(g[:, 0], 1)
        tot = int(g.sum())
        if best_tot is None or tot < best_tot:
            best, best_tot = blocks, tot
    return best


def kernel(x, edge_index, batch, conv_w1, conv_b1, conv_w2, conv_b2,
           lin1_w, lin1_b, lin2_w, lin2_b):
    n_graphs = 512
    # Uniform blocks: _pick_blocks' uneven split saves ~4% groups in the cost
    # model but measured slower on HW; uniform 25088 is the validated config.
    blocks = [25088] * 4
    in_maps, NT, G_tb, Np, gpg, core_graphs, order_tb = prep_inputs(
        x, edge_index, batch, conv_w1, conv_b1, conv_w2, conv_b2,
        lin1_w, lin1_b, lin2_w, lin2_b, n_graphs, blocks, 4)
    nc = build_program(NT, G_tb, Np, blocks, gpg, order_tb)
    res = run_bass_kernel_spmd(nc, in_maps, list(range(NCORES)),
                               trace=_trace["on"])
    _trace["last"] = res
    out = np.zeros((n_graphs, NCLS), np.float32)
    for c in range(NCORES):
        out[core_graphs[c]] = np.asarray(res.results[c]["out"], np.float32)
    return out

